# revision 28
# baseline (speedup 1.0000x reference)
# ContextQueryAttention (BiDAF-style) Trainium2 Bass/Tile kernel.
#
# Full-input contract: kernel(**inputs) takes the full arrays
#   context [32, 2048, 128] f32, query [32, 128, 128] f32,
#   w [384] f32, query_mask [32, 128] i32
# and returns out [32, 2048, 512] f32.
#
# Sharding: batch B=32 split 4-per-core across 8 NeuronCores (pure data
# parallel, no collectives).
#
# Math (per batch, C=2048, Q=128, D=128):
#   S[c,q] = ctx[c]@w1 + query[q]@w2 + (ctx[c]*w3)@query[q]
#          = alpha[c] + beta[q] + G[c,q]
#   a = softmax_q(S + maskadd);  c2q = a @ query
#   m[c] = max_q(S + maskadd);   b = softmax_c(m); q2c = b @ ctx
#   out = [ctx | c2q | ctx*c2q | ctx*q2c]
#
# This version is DMA-roofline oriented: the cost-model DMA device is
# exclusive at ~360 GB/s, and the mandatory traffic (4.25 MiB loads +
# 16 MiB stores per core) is ~59.4 us.  Everything else is arranged so
# the DMA queue never starves:
#  * All loads are issued up front (1 ctx DMA per batch into a per-batch
#    mega assembly tile [128, 16*512], plus one query DMA, one mask DMA,
#    3 w DMAs).  Four mega tiles stay resident, so no load waits on
#    stores.
#  * Stores: per group [128, 4, 384] (ctx|c2q|ctx*c2q) as soon as that
#    group's c2q columns are done, then [128, 4, 128] for ctx*q2c.
#  * alpha cancels in the row softmax; E^T = exp(G^T + beta') is
#    computed in [q, c] layout with beta' fused into the ACT exp bias.
#  * E is kept in bf16: c2q matmuls and E-transposes run at 1 cyc/row
#    on the PE.  Z (softmax denom) comes from an extra N=1 matmul with a
#    ones column (free on the PE).  u[d] = sum_c e_m[c]*ctx[c,d] is
#    N=1-per-tile with ctx as stationary (free on the PE).
#  * q2c normalization+broadcast: zb (all-partition scalar) via a
#    stride-0 ones matmul, rzb on DVE, bc = (u bcast)^T @ (ident*rzb)
#    gives rows of q2c/zb directly.
#  * Vector work is fused per group (3D APs) and split DVE/Pool/ACT.
#
# PSUM: 8 banks = tr 2 (ctx/E transposes) + pa 4 (qT, S^T, c2q, bc) +
# pz 1 (z cols, alpha, bcol, zb) + pu 1 (u accumulation chain).

import numpy as np

C = 2048
Q = 128
D = 128
B_TOTAL = 32
N_CORES = 8
B_LOCAL = B_TOTAL // N_CORES  # 4
N_CT = C // 128  # 16 c-tiles per batch
N_G = 4  # groups of 4 c-tiles

_compiled = None


def _build():
    import concourse.bacc as bacc
    import concourse.tile as tile
    import concourse.mybir as mybir
    from concourse import masks

    f32 = mybir.dt.float32
    i32 = mybir.dt.int32

    nc = bacc.Bacc(
        "TRN2",
        target_bir_lowering=False,
        debug=False,
        num_devices=N_CORES,
    )

    ctx_d = nc.dram_tensor("context", [B_LOCAL, C, D], f32, kind="ExternalInput").ap()
    qry_d = nc.dram_tensor("query", [B_LOCAL, Q, D], f32, kind="ExternalInput").ap()
    w_d = nc.dram_tensor("w", [3 * D], f32, kind="ExternalInput").ap()
    msk_d = nc.dram_tensor("query_mask", [B_LOCAL, Q], i32, kind="ExternalInput").ap()
    out_d = nc.dram_tensor("out", [B_LOCAL, C, 4 * D], f32, kind="ExternalOutput").ap()

    with tile.TileContext(nc) as tc:
        _kernel_body(tc, out_d, ctx_d, qry_d, w_d, msk_d, mybir, masks)

    nc.compile()
    return nc


def _kernel_body(tc, out_d, ctx_d, qry_d, w_d, msk_d, mybir, masks):
    from contextlib import ExitStack

    nc = tc.nc
    f32 = mybir.dt.float32
    f32r = mybir.dt.float32r
    bf16 = mybir.dt.bfloat16
    i32 = mybir.dt.int32
    AFT = mybir.ActivationFunctionType
    Alu = mybir.AluOpType
    AX = mybir.AxisListType.X

    es = ExitStack()
    with es:
        # ---- pools ----
        consts = es.enter_context(tc.tile_pool(name="consts", bufs=1))
        megas = es.enter_context(tc.tile_pool(name="megas", bufs=B_LOCAL))
        bigs = es.enter_context(tc.tile_pool(name="bigs", bufs=2))
        meds = es.enter_context(tc.tile_pool(name="meds", bufs=2))
        # PSUM (bank-granular): tr 2 + pa 4 + pz 1 + pu 1 = 8 banks.
        ps_tr = es.enter_context(tc.tile_pool(name="ps_tr", bufs=2, space="PSUM"))
        ps_a = es.enter_context(tc.tile_pool(name="ps_a", bufs=4, space="PSUM"))
        ps_z = es.enter_context(tc.tile_pool(name="ps_z", bufs=1, space="PSUM"))
        ps_u = es.enter_context(tc.tile_pool(name="ps_u", bufs=1, space="PSUM"))

        # ---- mega assembly tiles + all loads issued up front ----
        ctx_v = ctx_d.rearrange("b (j p) d -> b p j d", p=128)
        out_v = out_d.rearrange("b (j p) f -> b p j f", p=128)

        megam = []
        for b in range(B_LOCAL):
            mega = megas.tile([128, N_CT * 512], f32, tag="mega")
            megam.append(mega.rearrange("p (j f) -> p j f", j=N_CT))

        qall = consts.tile([128, B_LOCAL, 128], f32)
        mask4i = consts.tile([128, B_LOCAL], i32)
        wcols = consts.tile([128, 3], f32)
        w1_col = wcols[:, 0:1]
        w2_col = wcols[:, 1:2]
        w3_col = wcols[:, 2:3]

        # DMA issue order on the SP queue: query (gates qT, top of the b0
        # critical chain), ctx b0 group 0 (gates tr g0), w + mask, rest of
        # ctx b0, then ctx b1-3 whole.  Few, mostly-big DMAs keep the
        # exclusive DMA device saturated from the start.
        nc.sync.dma_start(out=qall[:], in_=qry_d.rearrange("b q d -> q b d"))
        nc.sync.dma_start(
            out=megam[0][:, 0:4, 0:128], in_=ctx_v[0][:, 0:4, :]
        )
        nc.sync.dma_start(out=wcols[:], in_=w_d.rearrange("(k d) -> d k", k=3))
        nc.sync.dma_start(
            out=megam[0][:, 4:16, 0:128], in_=ctx_v[0][:, 4:16, :]
        )
        # mask via the Pool SWDGE queue: slots into the DMA device between
        # HWDGE-fed loads without taking an HWDGE slot.
        nc.gpsimd.dma_start(out=mask4i[:], in_=msk_d.rearrange("b q -> q b"))
        for b in range(1, B_LOCAL):
            nc.sync.dma_start(out=megam[b][:, :, 0:128], in_=ctx_v[b])

        # ---- constants / globals ----
        wz = consts.tile([128, 128], f32)
        nc.vector.memset(wz[:], 0.0)
        ident = consts.tile([128, 128], f32)
        masks.make_identity(nc, ident[:])
        ident_bf = consts.tile([128, 128], bf16)
        nc.vector.tensor_copy(ident_bf[:], ident[:])
        ones_col = consts.tile([128, 1], f32)
        nc.vector.memset(ones_col[:], 1.0)
        ones_bf = consts.tile([128, 1], bf16)
        nc.vector.memset(ones_bf[:], 1.0)

        # query in bf16 (c2q moving operand), mask additive term
        qbf = consts.tile([128, B_LOCAL, 128], bf16)
        nc.vector.tensor_copy(
            qbf.rearrange("p b d -> p (b d)"), qall.rearrange("p b d -> p (b d)")
        )
        madd4 = consts.tile([128, B_LOCAL], f32)
        nc.gpsimd.tensor_copy(madd4[:], mask4i[:])  # int -> float cast
        nc.gpsimd.tensor_scalar(
            madd4[:], madd4[:], 1.0, 1.0e9, op0=Alu.subtract, op1=Alu.mult
        )

        # PE p-state warmup: the tensor engine needs ~3us of continuous work
        # to reach full clock.  Start dummy matmuls on a zeroed tile as early
        # as possible (before the identity is even built), sized so the ramp
        # completes right as the first ctx tile lands.
        pz0 = ps_z.tile([128, 512], f32, tag="pz")
        for _ in range(7):
            nc.tensor.matmul(pz0[:, 0:128], wz[:], wz[:], start=True, stop=True)

        for b in range(B_LOCAL):
            mega = megam[b]
            ctxT = bigs.tile([128, C], f32r, tag="ctxT")
            e_t = bigs.tile([128, C], bf16, tag="et")
            pz = pz0 if b == 0 else ps_z.tile([128, 512], f32, tag="pz")
            # pz layout: z 0:16 | alpha 16:32 | bcol 32:33 | zb 33:34

            # ---------- qT / beta prefix ----------
            qT_ps = ps_a.tile([128, 128], f32, tag="pa")
            nc.tensor.transpose(qT_ps[:], qall[:, b, :], ident[:])
            qT = meds.tile([128, 128], f32, tag="qT")
            nc.scalar.copy(qT[:], qT_ps[:])
            # qw3T[d, q] = qT * w3[d]  (ACT: keeps DVE free; f32r for S^T)
            qw3T = meds.tile([128, 128], f32r, tag="qw3T")
            nc.scalar.mul(qw3T[:], qT[:], w3_col[:])
            # beta' = qT@w2 + maskadd
            nc.tensor.matmul(pz[:, 32:33], qT[:], w2_col[:], start=True, stop=True)
            beta_col = meds.tile([128, 1], f32, tag="beta")

            # ---------- phase A: tr g -> copy g -> S^T g -> exp g ----------
            maxE = meds.tile([128, N_CT], f32, tag="maxE")
            e_m = meds.tile([128, N_CT], f32, tag="e_m")
            rz16 = meds.tile([128, N_CT], f32, tag="rz16")
            pu = ps_u.tile([128, 1], f32, tag="pu")

            nc.scalar.add(beta_col[:], pz[:, 32:33], madd4[:, b : b + 1])
            for g in range(N_G):
                tr = ps_tr.tile([128, 512], f32, tag="tr")
                for j in range(4):
                    nc.tensor.transpose(
                        tr[:, j * 128 : (j + 1) * 128], mega[:, 4 * g + j, 0:128],
                        ident[:],
                    )
                nc.scalar.copy(ctxT[:, g * 512 : (g + 1) * 512], tr[:])
                st = ps_a.tile([128, 512], f32, tag="pa")
                nc.tensor.matmul(
                    st[:], qw3T[:], ctxT[:, g * 512 : (g + 1) * 512],
                    start=True, stop=True,
                )
                nc.scalar.activation(
                    out=e_t[:, g * 512 : (g + 1) * 512], in_=st[:],
                    func=AFT.Exp, bias=beta_col[:], scale=1.0,
                )

            # ---------- phase B: c2q, Z, E-transpose, rowmax, outputs ------
            for g in range(N_G):
                cq = ps_a.tile([128, 512], f32, tag="pa")
                for j in range(4):
                    i = 4 * g + j
                    et_sl = e_t[:, i * 128 : (i + 1) * 128]
                    nc.tensor.matmul(
                        cq[:, j * 128 : (j + 1) * 128], et_sl, qbf[:, b, :],
                        start=True, stop=True,
                    )
                    nc.tensor.matmul(
                        pz[:, i : i + 1], et_sl, ones_bf[:], start=True, stop=True
                    )
                etr = ps_tr.tile([128, 512], bf16, tag="tr")
                for j in range(4):
                    i = 4 * g + j
                    nc.tensor.transpose(
                        etr[:, j * 128 : (j + 1) * 128],
                        e_t[:, i * 128 : (i + 1) * 128],
                        ident_bf[:],
                    )
                etr3 = etr.rearrange("p (j q) -> p j q", j=4)
                nc.vector.reduce_max(
                    out=maxE[:, 4 * g : 4 * g + 4], in_=etr3[:], axis=AX
                )
                # c2q = (E @ q) * (1/Z)
                nc.vector.reciprocal(
                    rz16[:, 4 * g : 4 * g + 4], pz[:, 4 * g : 4 * g + 4]
                )
                for j in range(4):
                    i = 4 * g + j
                    nc.vector.tensor_scalar_mul(
                        mega[:, i, 128:256],
                        cq[:, j * 128 : (j + 1) * 128],
                        rz16[:, i : i + 1],
                    )
                # out3 = ctx * c2q
                nc.gpsimd.tensor_tensor(
                    out=mega[:, 4 * g : 4 * g + 4, 256:384],
                    in0=mega[:, 4 * g : 4 * g + 4, 128:256],
                    in1=mega[:, 4 * g : 4 * g + 4, 0:128],
                    op=Alu.mult,
                )
                nc.sync.dma_start(
                    out=out_v[b, :, 4 * g : 4 * g + 4, 0:384],
                    in_=mega[:, 4 * g : 4 * g + 4, 0:384],
                )

            # ---------- batch tail: alpha, e_m, u ----------
            # alpha[c] = ctx@w1 (N=1, ~free); only feeds the q2c weights.
            for i in range(N_CT):
                nc.tensor.matmul(
                    pz[:, 16 + i : 17 + i],
                    ctxT[:, i * 128 : (i + 1) * 128].bitcast(f32),
                    w1_col[:],
                    start=True,
                    stop=True,
                )
            e_alpha = meds.tile([128, N_CT], f32, tag="e_alpha")
            nc.scalar.activation(out=e_alpha[:], in_=pz[:, 16:32], func=AFT.Exp)
            nc.vector.tensor_mul(e_m[:], e_alpha[:], maxE[:])
            # u[d] = sum_c e_m[c] ctx[c,d]: N=1 chain
            for i in range(N_CT):
                nc.tensor.matmul(
                    pu[:],
                    mega[:, i, 0:128],
                    e_m[:, i : i + 1],
                    start=(i == 0),
                    stop=(i == N_CT - 1),
                )

            # ---------- q2c epilogue ----------
            zsum = meds.tile([128, 1], f32, tag="zsum")
            nc.vector.reduce_sum(out=zsum[:], in_=e_m[:], axis=AX)
            # zb on every partition: ones[c,:]^T @ zsum
            nc.tensor.matmul(
                pz[:, 33:34], ones_col[:].broadcast_to((128, 128)), zsum[:],
                start=True, stop=True,
            )
            rzb = meds.tile([128, 1], f32, tag="rzb")
            nc.vector.reciprocal(rzb[:], pz[:, 33:34])
            u_sb = meds.tile([128, 1], f32, tag="u_sb")
            nc.scalar.copy(u_sb[:], pu[:])
            siT = meds.tile([128, 128], f32, tag="siT")
            nc.vector.tensor_scalar_mul(siT[:], ident[:], rzb[:])
            # bc[c, d] = sum_k u[k] * I[k, d] * rzb[k] = q2c[d] (all rows)
            bc = ps_a.tile([128, 128], f32, tag="pa")
            nc.tensor.matmul(
                bc[:], u_sb[:].broadcast_to((128, 128)), siT[:],
                start=True, stop=True,
            )
            bcv = bc.rearrange("p d -> p () d").broadcast_to((128, N_CT, 128))
            nc.vector.tensor_tensor(
                out=mega[:, :, 384:512], in0=mega[:, :, 0:128], in1=bcv,
                op=Alu.mult,
            )
            for g in range(N_G):
                nc.sync.dma_start(
                    out=out_v[b, :, 4 * g : 4 * g + 4, 384:512],
                    in_=mega[:, 4 * g : 4 * g + 4, 384:512],
                )


def kernel(**inputs):
    global _compiled
    from concourse.bass_utils import run_bass_kernel_spmd

    context = np.ascontiguousarray(inputs["context"], dtype=np.float32)
    query = np.ascontiguousarray(inputs["query"], dtype=np.float32)
    w = np.ascontiguousarray(inputs["w"], dtype=np.float32)
    qmask = np.ascontiguousarray(inputs["query_mask"], dtype=np.int32)

    if _compiled is None:
        _compiled = _build()
    nc = _compiled

    core_ids = list(range(N_CORES))
    in_maps = []
    for k in core_ids:
        sl = slice(k * B_LOCAL, (k + 1) * B_LOCAL)
        in_maps.append(
            {
                "context": context[sl],
                "query": query[sl],
                "w": w,
                "query_mask": qmask[sl],
            }
        )

    res = run_bass_kernel_spmd(nc, in_maps, core_ids)
    outs = [res.results[k]["out"] for k in range(N_CORES)]
    return np.concatenate(outs, axis=0)


# revision 34
# speedup vs baseline: 1.0080x; 1.0080x over previous
# ContextQueryAttention (BiDAF-style) Trainium2 Bass/Tile kernel.
#
# Full-input contract: kernel(**inputs) takes the full arrays
#   context [32, 2048, 128] f32, query [32, 128, 128] f32,
#   w [384] f32, query_mask [32, 128] i32
# and returns out [32, 2048, 512] f32.
#
# Sharding: batch B=32 split 4-per-core across 8 NeuronCores (pure data
# parallel, no collectives).
#
# Math (per batch, C=2048, Q=128, D=128):
#   S[c,q] = ctx[c]@w1 + query[q]@w2 + (ctx[c]*w3)@query[q]
#          = alpha[c] + beta[q] + G[c,q]
#   a = softmax_q(S + maskadd);  c2q = a @ query
#   m[c] = max_q(S + maskadd);   b = softmax_c(m); q2c = b @ ctx
#   out = [ctx | c2q | ctx*c2q | ctx*q2c]
#
# This version is DMA-roofline oriented: the cost-model DMA device is
# exclusive at ~360 GB/s, and the mandatory traffic (4.25 MiB loads +
# 16 MiB stores per core) is ~59.4 us.  Everything else is arranged so
# the DMA queue never starves:
#  * All loads are issued up front (1 ctx DMA per batch into a per-batch
#    mega assembly tile [128, 16*512], plus one query DMA, one mask DMA,
#    3 w DMAs).  Four mega tiles stay resident, so no load waits on
#    stores.
#  * Stores: per group [128, 4, 384] (ctx|c2q|ctx*c2q) as soon as that
#    group's c2q columns are done, then [128, 4, 128] for ctx*q2c.
#  * alpha cancels in the row softmax; E^T = exp(G^T + beta') is
#    computed in [q, c] layout with beta' fused into the ACT exp bias.
#  * E is kept in bf16: c2q matmuls and E-transposes run at 1 cyc/row
#    on the PE.  Z (softmax denom) comes from an extra N=1 matmul with a
#    ones column (free on the PE).  u[d] = sum_c e_m[c]*ctx[c,d] is
#    N=1-per-tile with ctx as stationary (free on the PE).
#  * q2c normalization+broadcast: zb (all-partition scalar) via a
#    stride-0 ones matmul, rzb on DVE, bc = (u bcast)^T @ (ident*rzb)
#    gives rows of q2c/zb directly.
#  * Vector work is fused per group (3D APs) and split DVE/Pool/ACT.
#
# PSUM: 8 banks = tr 2 (ctx/E transposes) + pa 4 (qT, S^T, c2q, bc) +
# pz 1 (z cols, alpha, bcol, zb) + pu 1 (u accumulation chain).

import numpy as np

C = 2048
Q = 128
D = 128
B_TOTAL = 32
N_CORES = 8
B_LOCAL = B_TOTAL // N_CORES  # 4
N_CT = C // 128  # 16 c-tiles per batch
N_G = 4  # groups of 4 c-tiles

_compiled = None


def _build():
    import concourse.bacc as bacc
    import concourse.tile as tile
    import concourse.mybir as mybir
    from concourse import masks

    f32 = mybir.dt.float32
    i32 = mybir.dt.int32

    nc = bacc.Bacc(
        "TRN2",
        target_bir_lowering=False,
        debug=False,
        num_devices=N_CORES,
    )

    ctx_d = nc.dram_tensor("context", [B_LOCAL, C, D], f32, kind="ExternalInput").ap()
    qry_d = nc.dram_tensor("query", [B_LOCAL, Q, D], f32, kind="ExternalInput").ap()
    w_d = nc.dram_tensor("w", [3 * D], f32, kind="ExternalInput").ap()
    msk_d = nc.dram_tensor("query_mask", [B_LOCAL, Q], i32, kind="ExternalInput").ap()
    out_d = nc.dram_tensor("out", [B_LOCAL, C, 4 * D], f32, kind="ExternalOutput").ap()

    with tile.TileContext(nc) as tc:
        _kernel_body(tc, out_d, ctx_d, qry_d, w_d, msk_d, mybir, masks)

    nc.compile()
    return nc


def _kernel_body(tc, out_d, ctx_d, qry_d, w_d, msk_d, mybir, masks):
    from contextlib import ExitStack

    nc = tc.nc
    f32 = mybir.dt.float32
    f32r = mybir.dt.float32r
    bf16 = mybir.dt.bfloat16
    i32 = mybir.dt.int32
    AFT = mybir.ActivationFunctionType
    Alu = mybir.AluOpType
    AX = mybir.AxisListType.X

    es = ExitStack()
    with es:
        # ---- pools ----
        consts = es.enter_context(tc.tile_pool(name="consts", bufs=1))
        megas = es.enter_context(tc.tile_pool(name="megas", bufs=B_LOCAL))
        bigs = es.enter_context(tc.tile_pool(name="bigs", bufs=2))
        meds = es.enter_context(tc.tile_pool(name="meds", bufs=2))
        # PSUM (bank-granular): tr 2 + pa 4 + pz 1 + pu 1 = 8 banks.
        ps_tr = es.enter_context(tc.tile_pool(name="ps_tr", bufs=2, space="PSUM"))
        ps_a = es.enter_context(tc.tile_pool(name="ps_a", bufs=4, space="PSUM"))
        ps_z = es.enter_context(tc.tile_pool(name="ps_z", bufs=1, space="PSUM"))
        ps_u = es.enter_context(tc.tile_pool(name="ps_u", bufs=1, space="PSUM"))

        # ---- mega assembly tiles + all loads issued up front ----
        ctx_v = ctx_d.rearrange("b (j p) d -> b p j d", p=128)
        out_v = out_d.rearrange("b (j p) f -> b p j f", p=128)

        megam = []
        for b in range(B_LOCAL):
            mega = megas.tile([128, N_CT * 512], f32, tag="mega")
            megam.append(mega.rearrange("p (j f) -> p j f", j=N_CT))

        qall = consts.tile([128, B_LOCAL, 128], f32)
        mask4i = consts.tile([128, B_LOCAL], i32)
        wcols = consts.tile([128, 3], f32)
        w1_col = wcols[:, 0:1]
        w2_col = wcols[:, 1:2]
        w3_col = wcols[:, 2:3]

        # DMA issue order on the SP queue: query (gates qT, top of the b0
        # critical chain), ctx b0 group 0 (gates tr g0), w + mask, rest of
        # ctx b0, then ctx b1-3 whole.  Few, mostly-big DMAs keep the
        # exclusive DMA device saturated from the start.
        nc.sync.dma_start(out=qall[:], in_=qry_d.rearrange("b q d -> q b d"))
        nc.sync.dma_start(
            out=megam[0][:, 0:4, 0:128], in_=ctx_v[0][:, 0:4, :]
        )
        nc.sync.dma_start(out=wcols[:], in_=w_d.rearrange("(k d) -> d k", k=3))
        nc.sync.dma_start(
            out=megam[0][:, 4:16, 0:128], in_=ctx_v[0][:, 4:16, :]
        )
        # mask via the Pool SWDGE queue: slots into the DMA device between
        # HWDGE-fed loads without taking an HWDGE slot.
        nc.gpsimd.dma_start(out=mask4i[:], in_=msk_d.rearrange("b q -> q b"))
        for b in range(1, B_LOCAL):
            nc.sync.dma_start(out=megam[b][:, :, 0:128], in_=ctx_v[b])

        # ---- constants / globals ----
        wz = consts.tile([128, 128], f32)
        nc.vector.memset(wz[:], 0.0)
        ident = consts.tile([128, 128], f32)
        masks.make_identity(nc, ident[:])
        ident_bf = consts.tile([128, 128], bf16)
        nc.vector.tensor_copy(ident_bf[:], ident[:])
        ones_col = consts.tile([128, 1], f32)
        nc.vector.memset(ones_col[:], 1.0)
        ones_bf = consts.tile([128, 1], bf16)
        nc.vector.memset(ones_bf[:], 1.0)

        # query in bf16 (c2q moving operand), mask additive term
        qbf = consts.tile([128, B_LOCAL, 128], bf16)
        nc.vector.tensor_copy(
            qbf.rearrange("p b d -> p (b d)"), qall.rearrange("p b d -> p (b d)")
        )
        madd4 = consts.tile([128, B_LOCAL], f32)
        nc.gpsimd.tensor_copy(madd4[:], mask4i[:])  # int -> float cast
        nc.gpsimd.tensor_scalar(
            madd4[:], madd4[:], 1.0, 1.0e9, op0=Alu.subtract, op1=Alu.mult
        )
        # alpha uses w3-scaled ctxT, so its weight vector is w1/w3
        w1w3 = consts.tile([128, 1], f32)
        nc.vector.reciprocal(w1w3[:], w3_col)
        nc.vector.tensor_mul(w1w3[:], w1w3[:], w1_col)

        # PE p-state warmup: the tensor engine needs ~3us of continuous work
        # to reach full clock.  Start dummy matmuls on a zeroed tile as early
        # as possible (before the identity is even built), sized so the ramp
        # completes right as the first ctx tile lands.
        pz0 = ps_z.tile([128, 512], f32, tag="pz")
        for _ in range(7):
            nc.tensor.matmul(pz0[:, 0:128], wz[:], wz[:], start=True, stop=True)

        for b in range(B_LOCAL):
            mega = megam[b]
            ctxT = bigs.tile([128, C], f32r, tag="ctxT")
            e_t = bigs.tile([128, C], bf16, tag="et")
            pz = pz0 if b == 0 else ps_z.tile([128, 512], f32, tag="pz")
            # pz layout: z 0:16 | alpha 16:32 | bcol 32:33 | zb 33:34

            # ---------- qT / beta prefix ----------
            # qT is the f32r stationary of the S^T matmuls; w3 is folded into
            # the ctxT copies instead (per-partition ACT scale), keeping qw3T
            # off the critical chain entirely.
            qT_ps = ps_a.tile([128, 128], f32, tag="pa")
            nc.tensor.transpose(qT_ps[:], qall[:, b, :], ident[:])
            qT = meds.tile([128, 128], f32r, tag="qT")
            nc.scalar.copy(qT[:], qT_ps[:])
            # beta' = qT@w2 + maskadd
            nc.tensor.matmul(
                pz[:, 32:33], qT[:].bitcast(f32), w2_col[:], start=True, stop=True
            )
            beta_col = meds.tile([128, 1], f32, tag="beta")

            # ---------- phase A: tr g -> copy g -> S^T g -> exp g ----------
            maxE = meds.tile([128, N_CT], f32, tag="maxE")
            e_m = meds.tile([128, N_CT], f32, tag="e_m")
            rz16 = meds.tile([128, N_CT], f32, tag="rz16")
            pu = ps_u.tile([128, 1], f32, tag="pu")

            nc.scalar.add(beta_col[:], pz[:, 32:33], madd4[:, b : b + 1])
            for g in range(N_G):
                tr = ps_tr.tile([128, 512], f32, tag="tr")
                for j in range(4):
                    nc.tensor.transpose(
                        tr[:, j * 128 : (j + 1) * 128], mega[:, 4 * g + j, 0:128],
                        ident[:],
                    )
                # ctxT = (ctx * w3)^T: w3 folded in as per-partition scale
                nc.scalar.mul(ctxT[:, g * 512 : (g + 1) * 512], tr[:], w3_col[:])
                st = ps_a.tile([128, 512], f32, tag="pa")
                nc.tensor.matmul(
                    st[:], qT[:], ctxT[:, g * 512 : (g + 1) * 512],
                    start=True, stop=True,
                )
                nc.scalar.activation(
                    out=e_t[:, g * 512 : (g + 1) * 512], in_=st[:],
                    func=AFT.Exp, bias=beta_col[:], scale=1.0,
                )

            # ---------- phase B: c2q, Z, E-transpose, rowmax, outputs ------
            for g in range(N_G):
                cq = ps_a.tile([128, 512], f32, tag="pa")
                for j in range(4):
                    i = 4 * g + j
                    et_sl = e_t[:, i * 128 : (i + 1) * 128]
                    nc.tensor.matmul(
                        cq[:, j * 128 : (j + 1) * 128], et_sl, qbf[:, b, :],
                        start=True, stop=True,
                    )
                    nc.tensor.matmul(
                        pz[:, i : i + 1], et_sl, ones_bf[:], start=True, stop=True
                    )
                etr = ps_tr.tile([128, 512], bf16, tag="tr")
                for j in range(4):
                    i = 4 * g + j
                    nc.tensor.transpose(
                        etr[:, j * 128 : (j + 1) * 128],
                        e_t[:, i * 128 : (i + 1) * 128],
                        ident_bf[:],
                    )
                etr3 = etr.rearrange("p (j q) -> p j q", j=4)
                nc.vector.reduce_max(
                    out=maxE[:, 4 * g : 4 * g + 4], in_=etr3[:], axis=AX
                )
                # c2q = (E @ q) * (1/Z)
                nc.vector.reciprocal(
                    rz16[:, 4 * g : 4 * g + 4], pz[:, 4 * g : 4 * g + 4]
                )
                for j in range(4):
                    i = 4 * g + j
                    nc.vector.tensor_scalar_mul(
                        mega[:, i, 128:256],
                        cq[:, j * 128 : (j + 1) * 128],
                        rz16[:, i : i + 1],
                    )
                # out3 = ctx * c2q
                nc.gpsimd.tensor_tensor(
                    out=mega[:, 4 * g : 4 * g + 4, 256:384],
                    in0=mega[:, 4 * g : 4 * g + 4, 128:256],
                    in1=mega[:, 4 * g : 4 * g + 4, 0:128],
                    op=Alu.mult,
                )
                nc.sync.dma_start(
                    out=out_v[b, :, 4 * g : 4 * g + 4, 0:384],
                    in_=mega[:, 4 * g : 4 * g + 4, 0:384],
                )

            # ---------- batch tail: alpha, e_m, u ----------
            # alpha[c] = ctx@w1 = (ctx*w3) @ (w1/w3)  (N=1, ~free)
            for i in range(N_CT):
                nc.tensor.matmul(
                    pz[:, 16 + i : 17 + i],
                    ctxT[:, i * 128 : (i + 1) * 128].bitcast(f32),
                    w1w3[:],
                    start=True,
                    stop=True,
                )
            e_alpha = meds.tile([128, N_CT], f32, tag="e_alpha")
            nc.scalar.activation(out=e_alpha[:], in_=pz[:, 16:32], func=AFT.Exp)
            nc.vector.tensor_mul(e_m[:], e_alpha[:], maxE[:])
            # u[d] = sum_c e_m[c] ctx[c,d]: N=1 chain
            for i in range(N_CT):
                nc.tensor.matmul(
                    pu[:],
                    mega[:, i, 0:128],
                    e_m[:, i : i + 1],
                    start=(i == 0),
                    stop=(i == N_CT - 1),
                )

            # ---------- q2c epilogue ----------
            zsum = meds.tile([128, 1], f32, tag="zsum")
            nc.vector.reduce_sum(out=zsum[:], in_=e_m[:], axis=AX)
            # zb on every partition: ones[c,:]^T @ zsum
            nc.tensor.matmul(
                pz[:, 33:34], ones_col[:].broadcast_to((128, 128)), zsum[:],
                start=True, stop=True,
            )
            rzb = meds.tile([128, 1], f32, tag="rzb")
            nc.vector.reciprocal(rzb[:], pz[:, 33:34])
            u_sb = meds.tile([128, 1], f32, tag="u_sb")
            nc.scalar.copy(u_sb[:], pu[:])
            siT = meds.tile([128, 128], f32, tag="siT")
            nc.vector.tensor_scalar_mul(siT[:], ident[:], rzb[:])
            # bc[c, d] = sum_k u[k] * I[k, d] * rzb[k] = q2c[d] (all rows)
            bc = ps_a.tile([128, 128], f32, tag="pa")
            nc.tensor.matmul(
                bc[:], u_sb[:].broadcast_to((128, 128)), siT[:],
                start=True, stop=True,
            )
            bcv = bc.rearrange("p d -> p () d").broadcast_to((128, N_CT, 128))
            nc.vector.tensor_tensor(
                out=mega[:, :, 384:512], in0=mega[:, :, 0:128], in1=bcv,
                op=Alu.mult,
            )
            for g in range(N_G):
                nc.sync.dma_start(
                    out=out_v[b, :, 4 * g : 4 * g + 4, 384:512],
                    in_=mega[:, 4 * g : 4 * g + 4, 384:512],
                )


def kernel(**inputs):
    global _compiled
    from concourse.bass_utils import run_bass_kernel_spmd

    context = np.ascontiguousarray(inputs["context"], dtype=np.float32)
    query = np.ascontiguousarray(inputs["query"], dtype=np.float32)
    w = np.ascontiguousarray(inputs["w"], dtype=np.float32)
    qmask = np.ascontiguousarray(inputs["query_mask"], dtype=np.int32)

    if _compiled is None:
        _compiled = _build()
    nc = _compiled

    core_ids = list(range(N_CORES))
    in_maps = []
    for k in core_ids:
        sl = slice(k * B_LOCAL, (k + 1) * B_LOCAL)
        in_maps.append(
            {
                "context": context[sl],
                "query": query[sl],
                "w": w,
                "query_mask": qmask[sl],
            }
        )

    res = run_bass_kernel_spmd(nc, in_maps, core_ids)
    outs = [res.results[k]["out"] for k in range(N_CORES)]
    return np.concatenate(outs, axis=0)
